# revision 31
# baseline (speedup 1.0000x reference)
"""Trainium2 Bass kernel for nn_Aggregator (gnn_message_passing).

Computation (see reference):
  entity_agg = segment_mean(entity_emb[tail] * weight[edge_type-2], head)
  user_agg   = 2 * (ua_interact_mat @ aspect_emb)   # softmax(...).sum(axis=1) == 1
  item_agg   = 2 * (ia_interact_mat @ aspect_emb)

Strategy (8 NeuronCores, SPMD, no collectives):
  - Entities are assigned to 128-entity "tiles" (balanced by degree with a
    snake pack).  Each core owns TPC tiles; edges are routed to the core
    owning their head entity.
  - The entity table is viewed as PAIRS [N/2, 128] (512B rows) so the random
    gather uses 512B descriptors (2x faster than 256B on trn2 SDMA).  Each
    edge gathers its tail's pair row with dma_gather (int16 indices, rebased
    per <=32768-row table segment; 2 equal segments).
  - The half-select (tail&1) is folded into the relation one-hot: a 32-row
    extended weight table w2[rel + 16*(tail&1)] whose low/high 64 channels
    hold weight[rel] in the selected half and zeros in the other.  Per-chunk:
      rel128 = onehot32T.T @ w2          (TensorE, K=32, fp16)
      msg128 = gathered_pair * rel128    (VectorE)
      sums128[w] += M.T @ msg128         (TensorE; M[e,w] one-hot on head slot)
    and per tile: entity_agg_slot = (sums128[:, :64] + sums128[:, 64:]) * recip.
  - Edges are grouped by (block, tail-segment, tile) with every (tile, seg)
    padded to L*128 slots -> fully static SPMD program.  Division by edge
    counts uses host-precomputed reciprocals; output rows are written in slot
    order and un-permuted on the host.
  - user/item path: row-parallel matmuls with host-transposed interact mats.
"""

import numpy as np

# ---- problem constants (hardcoded per harness contract) ----
N_ENTITIES = 100000
N_USERS = 100000
N_ITEMS = 50000
CHANNEL = 64
N_EDGES = 1250000
N_REL = 16
N_CORES = 8


def full_cfg():
    return dict(
        n_entities=N_ENTITIES,
        n_users=N_USERS,
        n_items=N_ITEMS,
        n_cores=N_CORES,
        seg=25000,           # pair-table rows per gather segment (<=32767)
        nseg=2,              # ceil(50000 pair rows / seg)
        tpc=98,              # entity tiles per core (98*128*8 = 100352 slots)
        bt=2,                # tiles per block
        nb=49,               # blocks per core (tpc = bt*nb)
        tu=98,               # user tiles per core  (98*128 >= 12500)
        ti=49,               # item tiles per core  (49*128 >= 6250)
        ua_grp=7,            # user tiles per DMA/compute group
        sub=4,               # chunks per PSUM-rel subgroup (bank limit)
    )


# ---------------------------------------------------------------------------
# host-side prep
# ---------------------------------------------------------------------------

def _snake_pack(deg, cfg):
    """Assign entities to tiles so per-tile degree sums are ~equal."""
    ne = cfg["n_entities"]
    t_total = cfg["n_cores"] * cfg["tpc"]
    s_total = t_total * 128
    order = np.argsort(-deg, kind="stable")
    order_p = np.full(s_total, -1, dtype=np.int64)
    order_p[:ne] = order
    rounds = order_p.reshape(128, t_total).copy()
    rounds[1::2] = rounds[1::2, ::-1]
    ent_of_slot = rounds.T.reshape(-1).copy()
    slot_of_ent = np.empty(ne, dtype=np.int64)
    valid = ent_of_slot >= 0
    slot_of_ent[ent_of_slot[valid]] = np.nonzero(valid)[0]
    return ent_of_slot, slot_of_ent


def host_prep(entity_emb, item_emb, user_emb, aspect_emb, edge_index, edge_type,
              ua_interact_mat, ia_interact_mat, weight, cfg):
    nc_ = cfg["n_cores"]
    ne = cfg["n_entities"]
    seg_sz = cfg["seg"]
    nseg = cfg["nseg"]
    tpc, bt, nb = cfg["tpc"], cfg["bt"], cfg["nb"]
    assert tpc == bt * nb
    npair = ne // 2
    assert nseg * seg_sz >= npair

    head = np.asarray(edge_index[0], dtype=np.int64)
    tail = np.asarray(edge_index[1], dtype=np.int64)
    rel = np.asarray(edge_type, dtype=np.int64) - 2
    E = head.shape[0]

    deg = np.bincount(head, minlength=ne).astype(np.int64)
    ent_of_slot, slot_of_ent = _snake_pack(deg, cfg)

    hslot = slot_of_ent[head]
    tile_g = hslot >> 7
    wslot = (hslot & 127).astype(np.float32)
    pair = tail >> 1
    seg = pair // seg_sz
    relx = rel + N_REL * (tail & 1)          # extended relation id (0..31)
    blk = (tile_g % tpc) // bt
    tloc = tile_g % bt
    core = tile_g // tpc

    n_groups = nc_ * nb * nseg * bt
    gid = ((core * nb + blk) * nseg + seg) * bt + tloc
    counts = np.bincount(gid, minlength=n_groups)
    L = max(1, -(-int(counts.max()) // 128))
    rnt = L * 128
    rn = bt * rnt
    epad = nb * nseg * bt * rnt
    nch = epad // 128

    sort_idx = np.argsort(gid, kind="stable")
    starts = np.zeros(n_groups + 1, dtype=np.int64)
    starts[1:] = np.cumsum(counts)
    rank = np.arange(E, dtype=np.int64) - starts[gid[sort_idx]]
    token = gid[sort_idx] * rnt + rank

    idx_flat = np.zeros(nc_ * epad, dtype=np.int16)
    idx_flat[token] = (pair - seg * seg_sz)[sort_idx].astype(np.int16)
    wv_flat = np.zeros(nc_ * epad, dtype=np.float32)
    wv_flat[token] = wslot[sort_idx]
    one_flat = np.zeros((2 * N_REL, nc_ * epad), dtype=np.float16)
    one_flat[relx[sort_idx], token] = np.float16(1.0)

    idx_r = idx_flat.reshape(nc_, nb, nseg, rn // 16, 16)
    idx_dev = np.ascontiguousarray(
        np.broadcast_to(
            idx_r.transpose(0, 1, 2, 4, 3)[:, :, :, None, :, :],
            (nc_, nb, nseg, 8, 16, rn // 16),
        ).reshape(nc_, nb, nseg, 128, rn // 16)
    )
    wv_dev = np.ascontiguousarray(
        wv_flat.reshape(nc_, nch, 128).transpose(0, 2, 1))          # [C,128,nch]
    one_dev = np.ascontiguousarray(
        one_flat.reshape(2 * N_REL, nc_, epad).transpose(1, 0, 2))  # [C,32,epad]

    dos = np.where(ent_of_slot >= 0, deg[np.maximum(ent_of_slot, 0)], 0)
    recip = (1.0 / np.maximum(dos, 1.0)).astype(np.float32)
    recip_dev = np.ascontiguousarray(
        recip.reshape(nc_, tpc, 128).transpose(0, 2, 1))            # [C,128,tpc]

    nu, ni = cfg["n_users"], cfg["n_items"]
    upc, ipc = nu // nc_, ni // nc_
    up, ip = cfg["tu"] * 128, cfg["ti"] * 128
    uaT = np.asarray(ua_interact_mat, dtype=np.float32).T
    iaT = np.asarray(ia_interact_mat, dtype=np.float32).T
    uaT_dev = np.zeros((nc_, CHANNEL, up), dtype=np.float32)
    iaT_dev = np.zeros((nc_, CHANNEL, ip), dtype=np.float32)
    for c in range(nc_):
        uaT_dev[c, :, :upc] = uaT[:, c * upc:(c + 1) * upc]
        iaT_dev[c, :, :ipc] = iaT[:, c * ipc:(c + 1) * ipc]

    iota = np.broadcast_to(np.arange(128, dtype=np.float32), (128, 128)).copy()
    # extended weight table: w2[r, 0:64] = weight[r] (tail even),
    #                        w2[16+r, 64:128] = weight[r] (tail odd)
    # hi+lo fp16 split of the fp32 weights: w ~= w_hi + w_lo exactly enough
    wf = np.asarray(weight, dtype=np.float32)
    w_hi = wf.astype(np.float16)
    w_lo = (wf - w_hi.astype(np.float32)).astype(np.float16)
    w2 = np.zeros((2, 2 * N_REL, 2 * CHANNEL), dtype=np.float16)
    for i, wc in enumerate((w_hi, w_lo)):
        w2[i, :N_REL, :CHANNEL] = wc
        w2[i, N_REL:, CHANNEL:] = wc
    aspect2 = (2.0 * np.asarray(aspect_emb, dtype=np.float32))
    # entity table as pair rows [npair(+pad), 128]
    ent_pairs = np.zeros((nseg * seg_sz, 2 * CHANNEL), dtype=np.float32)
    ent_pairs[:npair] = np.asarray(entity_emb, dtype=np.float32).reshape(npair, 2 * CHANNEL)

    in_maps = []
    for c in range(nc_):
        in_maps.append({
            "entp": ent_pairs,
            "idx16": np.ascontiguousarray(idx_dev[c]),
            "wvals": wv_dev[c],
            "oneT": one_dev[c],
            "recip": recip_dev[c],
            "iota": iota,
            "w2": w2,
            "aspect2": aspect2,
            "uaT": uaT_dev[c],
            "iaT": iaT_dev[c],
        })
    meta = dict(L=L, ent_of_slot=ent_of_slot, cfg=cfg)
    return in_maps, meta


# ---------------------------------------------------------------------------
# device program
# ---------------------------------------------------------------------------

def build_program(cfg, L, reps=1, parts=("gather", "compute", "aux")):
    import concourse.bacc as bacc
    import concourse.mybir as mybir
    from concourse.tile import TileContext

    f32 = mybir.dt.float32
    f16 = mybir.dt.float16
    i16 = mybir.dt.int16

    seg_sz = cfg["seg"]
    nseg = cfg["nseg"]
    tpc, bt, nb = cfg["tpc"], cfg["bt"], cfg["nb"]
    tu, ti, ua_grp = cfg["tu"], cfg["ti"], cfg["ua_grp"]
    sub = cfg["sub"]
    rnt = L * 128
    rn = bt * rnt
    epad = nb * nseg * bt * rnt
    nch = epad // 128
    up, ip = tu * 128, ti * 128
    C2 = 2 * CHANNEL

    nc = bacc.Bacc(None, target_bir_lowering=False, debug=False,
                   dynamic_dma_scratch_size=cfg.get("dma_scratch", 16384))
    entp = nc.declare_dram_parameter("entp", [nseg * seg_sz, C2], f32, isOutput=False)
    idx16 = nc.declare_dram_parameter("idx16", [nb, nseg, 128, rn // 16], i16, isOutput=False)
    wvals = nc.declare_dram_parameter("wvals", [128, nch], f32, isOutput=False)
    oneT = nc.declare_dram_parameter("oneT", [2 * N_REL, epad], f16, isOutput=False)
    recip = nc.declare_dram_parameter("recip", [128, tpc], f32, isOutput=False)
    iota = nc.declare_dram_parameter("iota", [128, 128], f32, isOutput=False)
    w2 = nc.declare_dram_parameter("w2", [2, 2 * N_REL, C2], f16, isOutput=False)
    aspect2 = nc.declare_dram_parameter("aspect2", [CHANNEL, CHANNEL], f32, isOutput=False)
    uaT = nc.declare_dram_parameter("uaT", [CHANNEL, up], f32, isOutput=False)
    iaT = nc.declare_dram_parameter("iaT", [CHANNEL, ip], f32, isOutput=False)
    ent_out = nc.declare_dram_parameter("ent_out", [tpc * 128, CHANNEL], f32, isOutput=True)
    ua_out = nc.declare_dram_parameter("ua_out", [up, CHANNEL], f32, isOutput=True)
    ia_out = nc.declare_dram_parameter("ia_out", [ip, CHANNEL], f32, isOutput=True)

    is_eq = mybir.AluOpType.is_equal
    mult = mybir.AluOpType.mult
    add = mybir.AluOpType.add

    ua_blocks = [("ua", g) for g in range(tu // ua_grp)]
    ia_blocks = [("ia", g) for g in range(ti // ua_grp)]
    aux = ua_blocks + ia_blocks

    with TileContext(nc) as tc:
        with (
            tc.tile_pool(name="const", bufs=1) as constp,
            tc.tile_pool(name="gbuf", bufs=3) as gbufp,
            tc.tile_pool(name="idxp", bufs=3) as idxp,
            tc.tile_pool(name="onep", bufs=3) as onep,
            tc.tile_pool(name="wvp", bufs=3) as wvp,
            tc.tile_pool(name="mbuf", bufs=4) as mbufp,
            tc.tile_pool(name="msgb", bufs=4) as msgp,
            tc.tile_pool(name="foldb", bufs=3) as foldp,
            tc.tile_pool(name="outp", bufs=2) as outp,
            tc.tile_pool(name="uain", bufs=2) as uainp,
            tc.tile_pool(name="uast", bufs=2) as uastp,
            tc.tile_pool(name="psrel", bufs=4, space="PSUM") as psrelp,
            tc.tile_pool(name="pssum", bufs=2, space="PSUM") as pssump,
            tc.tile_pool(name="psua", bufs=2, space="PSUM") as psuap,
        ):
            iota_sb = constp.tile([128, 128], f32)
            nc.sync.dma_start(out=iota_sb[:], in_=iota[:])
            w2_sb = constp.tile([2 * N_REL, 2, C2], f16)
            for i in range(2):
                nc.sync.dma_start(out=w2_sb[:, i, :], in_=w2[i])
            asp_sb = constp.tile([CHANNEL, CHANNEL], f32)
            nc.sync.dma_start(out=asp_sb[:], in_=aspect2[:])
            recip_sb = constp.tile([128, tpc], f32)
            nc.sync.dma_start(out=recip_sb[:], in_=recip[:])

            def emit_aux(kind, g):
                src = uaT if kind == "ua" else iaT
                dst = ua_out if kind == "ua" else ia_out
                cols = ua_grp * 128
                in_t = uainp.tile([CHANNEL, cols], f32, tag="uain")
                nc.sync.dma_start(out=in_t[:], in_=src[:, g * cols:(g + 1) * cols])
                stage = uastp.tile([128, ua_grp, CHANNEL], f32, tag="uast")
                for t in range(ua_grp):
                    ps = psuap.tile([128, CHANNEL], f32)
                    nc.tensor.matmul(
                        out=ps[:],
                        lhsT=in_t[:, t * 128:(t + 1) * 128],
                        rhs=asp_sb[:],
                        start=True, stop=True,
                    )
                    nc.vector.tensor_copy(out=stage[:, t, :], in_=ps[:])
                dview = dst[g * cols:(g + 1) * cols, :].rearrange(
                    "(a p) c -> p a c", p=128)
                nc.sync.dma_start(out=dview, in_=stage[:])

            n_aux_total = len(aux) * reps
            n_blk_total = nb * reps
            for rep_b in range(n_blk_total):
                b = rep_b % nb
                g_tile = gbufp.tile([128, nseg * bt * L, C2], f32, tag="gt")
                idx_t = idxp.tile([128, nseg, rn // 16], i16, tag="ix")
                one_t = onep.tile([2 * N_REL, nseg * bt * rnt], f16, tag="oh")
                wv_t = wvp.tile([128, nseg * bt * L], f32, tag="wv")
                for s in range(nseg):
                    nc.sync.dma_start(out=idx_t[:, s, :], in_=idx16[b, s])
                nc.sync.dma_start(out=one_t[:], in_=oneT[:, b * rn * nseg:(b + 1) * rn * nseg])
                nc.sync.dma_start(out=wv_t[:], in_=wvals[:, b * nseg * bt * L:(b + 1) * nseg * bt * L])
                for s in range(nseg if "gather" in parts else 0):
                    lo = s * seg_sz
                    for h in range(bt):
                        nc.gpsimd.dma_gather(
                            out_ap=g_tile[:, (s * bt + h) * L:(s * bt + h + 1) * L, :],
                            in_ap=entp[lo:lo + seg_sz, :],
                            idxs_ap=idx_t[:, s, h * (rnt // 16):(h + 1) * (rnt // 16)],
                            num_idxs=rnt,
                            num_idxs_reg=rnt,
                            elem_size=C2,
                            single_packet=False,
                        )
                for t in range(bt if "compute" in parts else 0):
                    ps_sum = pssump.tile([128, C2], f32)
                    for s in range(nseg):
                        g0 = s * bt * L + t * L
                        for c0 in range(0, L, sub):
                            nchk = min(sub, L - c0)
                            ps_rel = psrelp.tile([128, sub * C2], f32, tag="psrel")
                            m_t = mbufp.tile([128, sub, 128], f32, tag="m")
                            msg_t = msgp.tile([128, sub, C2], f32, tag="msg")
                            nc.any.tensor_tensor(
                                out=m_t[:, :nchk, :],
                                in0=wv_t[:, g0 + c0:g0 + c0 + nchk].unsqueeze(2).broadcast_to([128, nchk, 128]),
                                in1=iota_sb[:].unsqueeze(1).broadcast_to([128, nchk, 128]),
                                op=is_eq,
                            )
                            for c in range(nchk):
                                for i in range(2):
                                    nc.tensor.matmul(
                                        out=ps_rel[:, c * C2:(c + 1) * C2],
                                        lhsT=one_t[:, (g0 + c0 + c) * 128:(g0 + c0 + c + 1) * 128],
                                        rhs=w2_sb[:, i, :],
                                        start=(i == 0), stop=(i == 1),
                                        skip_group_check=True,
                                    )
                            nc.vector.tensor_tensor(
                                out=msg_t[:, :nchk, :],
                                in0=g_tile[:, g0 + c0:g0 + c0 + nchk, :],
                                in1=ps_rel[:, :nchk * C2].rearrange("p (l c) -> p l c", c=C2),
                                op=mult,
                            )
                            for c in range(nchk):
                                nc.tensor.matmul(
                                    out=ps_sum[:],
                                    lhsT=m_t[:, c, :],
                                    rhs=msg_t[:, c, :],
                                    start=(s == 0 and c0 + c == 0),
                                    stop=(s == nseg - 1 and c0 + c == L - 1),
                                    skip_group_check=True,
                                )
                    tg = b * bt + t
                    o_col = tg % ua_grp
                    if o_col == 0:
                        o_stage = outp.tile([128, ua_grp, CHANNEL], f32, tag="eo")
                    fold_t = foldp.tile([128, CHANNEL], f32, tag="fold")
                    nc.vector.tensor_scalar(
                        out=fold_t[:],
                        in0=ps_sum[:, :CHANNEL],
                        scalar1=recip_sb[:, tg:tg + 1],
                        scalar2=None,
                        op0=mult,
                    )
                    nc.vector.scalar_tensor_tensor(
                        out=o_stage[:, o_col, :],
                        in0=ps_sum[:, CHANNEL:],
                        scalar=recip_sb[:, tg:tg + 1],
                        in1=fold_t[:],
                        op0=mult,
                        op1=add,
                    )
                    if o_col == ua_grp - 1:
                        blk0 = tg - (ua_grp - 1)
                        oview = ent_out[blk0 * 128:(tg + 1) * 128, :].rearrange(
                            "(a p) c -> p a c", p=128)
                        nc.sync.dma_start(out=oview, in_=o_stage[:])
                for j in range(rep_b * n_aux_total // n_blk_total,
                               (rep_b + 1) * n_aux_total // n_blk_total):
                    if "aux" in parts:
                        emit_aux(*aux[j % len(aux)])

    nc.compile()
    return nc


# ---------------------------------------------------------------------------
# entry point
# ---------------------------------------------------------------------------

def kernel(entity_emb, item_emb, user_emb, aspect_emb, edge_index, edge_type,
           ua_interact_mat, ia_interact_mat, weight, _cfg=None, _run_opts=None):
    from concourse.bass_utils import run_bass_kernel_spmd

    cfg = _cfg or full_cfg()
    in_maps, meta = host_prep(
        entity_emb, item_emb, user_emb, aspect_emb, edge_index, edge_type,
        ua_interact_mat, ia_interact_mat, weight, cfg)
    nc = build_program(cfg, meta["L"])

    res = run_bass_kernel_spmd(nc, in_maps, list(range(cfg["n_cores"])),
                               **(_run_opts or {}))
    outs = res.results

    nc_ = cfg["n_cores"]
    ne, nu, ni = cfg["n_entities"], cfg["n_users"], cfg["n_items"]
    upc, ipc = nu // nc_, ni // nc_
    ent_of_slot = meta["ent_of_slot"]

    ent_stack = np.concatenate([outs[c]["ent_out"] for c in range(nc_)], axis=0)
    entity_agg = np.zeros((ne, CHANNEL), dtype=np.float32)
    valid = ent_of_slot >= 0
    entity_agg[ent_of_slot[valid]] = ent_stack[valid]

    user_agg = np.concatenate([outs[c]["ua_out"][:upc] for c in range(nc_)], axis=0)
    item_agg = np.concatenate([outs[c]["ia_out"][:ipc] for c in range(nc_)], axis=0)

    kernel._last_results = res
    return (item_agg, entity_agg, user_agg)


# revision 34
# speedup vs baseline: 6.2825x; 6.2825x over previous
"""Trainium2 Bass kernel for nn_Aggregator (gnn_message_passing).

Computation (see reference):
  entity_agg = segment_mean(entity_emb[tail] * weight[edge_type-2], head)
  user_agg   = 2 * (ua_interact_mat @ aspect_emb)   # softmax(...).sum(axis=1) == 1
  item_agg   = 2 * (ia_interact_mat @ aspect_emb)

Strategy (8 NeuronCores, SPMD, no collectives):
  - Entities are assigned to 128-entity "tiles" (balanced by degree with a
    snake pack).  Each core owns TPC tiles; edges are routed to the core
    owning their head entity.
  - The entity table is viewed as PAIRS [N/2, 128] (512B rows) so the random
    gather uses 512B descriptors (2x faster than 256B on trn2 SDMA).  Each
    edge gathers its tail's pair row with dma_gather (int16 indices, rebased
    per <=32768-row table segment; 2 equal segments).
  - The half-select (tail&1) is folded into the relation one-hot: a 32-row
    extended weight table w2[rel + 16*(tail&1)] whose low/high 64 channels
    hold weight[rel] in the selected half and zeros in the other.  Per-chunk:
      rel128 = onehot32T.T @ w2          (TensorE, K=32, fp16)
      msg128 = gathered_pair * rel128    (VectorE)
      sums128[w] += M.T @ msg128         (TensorE; M[e,w] one-hot on head slot)
    and per tile: entity_agg_slot = (sums128[:, :64] + sums128[:, 64:]) * recip.
  - Edges are grouped by (block, tail-segment, tile) with every (tile, seg)
    padded to L*128 slots -> fully static SPMD program.  Division by edge
    counts uses host-precomputed reciprocals; output rows are written in slot
    order and un-permuted on the host.
  - user/item path: row-parallel matmuls with host-transposed interact mats.
"""

import numpy as np

# ---- problem constants (hardcoded per harness contract) ----
N_ENTITIES = 100000
N_USERS = 100000
N_ITEMS = 50000
CHANNEL = 64
N_EDGES = 1250000
N_REL = 16
N_CORES = 8


def full_cfg():
    return dict(
        n_entities=N_ENTITIES,
        n_users=N_USERS,
        n_items=N_ITEMS,
        n_cores=N_CORES,
        seg=25000,           # pair-table rows per gather segment (<=32767)
        nseg=2,              # ceil(50000 pair rows / seg)
        tpc=98,              # entity tiles per core (98*128*8 = 100352 slots)
        bt=2,                # tiles per block
        nb=49,               # blocks per core (tpc = bt*nb)
        tu=98,               # user tiles per core  (98*128 >= 12500)
        ti=49,               # item tiles per core  (49*128 >= 6250)
        ua_grp=7,            # user tiles per DMA/compute group
        sub=4,               # chunks per PSUM-rel subgroup (bank limit)
    )


# ---------------------------------------------------------------------------
# host-side prep
# ---------------------------------------------------------------------------

def _snake_pack(deg, cfg):
    """Assign entities to tiles so per-tile degree sums are ~equal."""
    ne = cfg["n_entities"]
    t_total = cfg["n_cores"] * cfg["tpc"]
    s_total = t_total * 128
    order = np.argsort(-deg, kind="stable")
    order_p = np.full(s_total, -1, dtype=np.int64)
    order_p[:ne] = order
    rounds = order_p.reshape(128, t_total).copy()
    rounds[1::2] = rounds[1::2, ::-1]
    ent_of_slot = rounds.T.reshape(-1).copy()
    slot_of_ent = np.empty(ne, dtype=np.int64)
    valid = ent_of_slot >= 0
    slot_of_ent[ent_of_slot[valid]] = np.nonzero(valid)[0]
    return ent_of_slot, slot_of_ent


def host_prep(entity_emb, item_emb, user_emb, aspect_emb, edge_index, edge_type,
              ua_interact_mat, ia_interact_mat, weight, cfg):
    nc_ = cfg["n_cores"]
    ne = cfg["n_entities"]
    seg_sz = cfg["seg"]
    nseg = cfg["nseg"]
    tpc, bt, nb = cfg["tpc"], cfg["bt"], cfg["nb"]
    assert tpc == bt * nb
    npair = ne // 2
    assert nseg * seg_sz >= npair

    head = np.asarray(edge_index[0], dtype=np.int64)
    tail = np.asarray(edge_index[1], dtype=np.int64)
    rel = np.asarray(edge_type, dtype=np.int64) - 2
    E = head.shape[0]

    deg = np.bincount(head, minlength=ne).astype(np.int64)
    ent_of_slot, slot_of_ent = _snake_pack(deg, cfg)

    hslot = slot_of_ent[head]
    tile_g = hslot >> 7
    wslot = (hslot & 127).astype(np.float32)
    pair = tail >> 1
    seg = pair // seg_sz
    relx = rel + N_REL * (tail & 1)          # extended relation id (0..31)
    blk = (tile_g % tpc) // bt
    tloc = tile_g % bt
    core = tile_g // tpc

    n_groups = nc_ * nb * nseg * bt
    gid = ((core * nb + blk) * nseg + seg) * bt + tloc
    counts = np.bincount(gid, minlength=n_groups)
    L = max(1, -(-int(counts.max()) // 128))
    rnt = L * 128
    rn = bt * rnt
    epad = nb * nseg * bt * rnt
    nch = epad // 128

    sort_idx = np.argsort(gid, kind="stable")
    starts = np.zeros(n_groups + 1, dtype=np.int64)
    starts[1:] = np.cumsum(counts)
    rank = np.arange(E, dtype=np.int64) - starts[gid[sort_idx]]
    token = gid[sort_idx] * rnt + rank

    idx_flat = np.zeros(nc_ * epad, dtype=np.int16)
    idx_flat[token] = (pair - seg * seg_sz)[sort_idx].astype(np.int16)
    wv_flat = np.zeros(nc_ * epad, dtype=np.float32)
    wv_flat[token] = wslot[sort_idx]
    one_flat = np.zeros((2 * N_REL, nc_ * epad), dtype=np.float16)
    one_flat[relx[sort_idx], token] = np.float16(1.0)

    idx_r = idx_flat.reshape(nc_, nb, nseg, rn // 16, 16)
    idx_dev = np.ascontiguousarray(
        np.broadcast_to(
            idx_r.transpose(0, 1, 2, 4, 3)[:, :, :, None, :, :],
            (nc_, nb, nseg, 8, 16, rn // 16),
        ).reshape(nc_, nb, nseg, 128, rn // 16)
    )
    wv_dev = np.ascontiguousarray(
        wv_flat.reshape(nc_, nch, 128).transpose(0, 2, 1))          # [C,128,nch]
    one_dev = np.ascontiguousarray(
        one_flat.reshape(2 * N_REL, nc_, epad).transpose(1, 0, 2))  # [C,32,epad]

    dos = np.where(ent_of_slot >= 0, deg[np.maximum(ent_of_slot, 0)], 0)
    recip = (1.0 / np.maximum(dos, 1.0)).astype(np.float32)
    recip_dev = np.ascontiguousarray(
        recip.reshape(nc_, tpc, 128).transpose(0, 2, 1))            # [C,128,tpc]

    nu, ni = cfg["n_users"], cfg["n_items"]
    upc, ipc = nu // nc_, ni // nc_
    up, ip = cfg["tu"] * 128, cfg["ti"] * 128
    uaT = np.asarray(ua_interact_mat, dtype=np.float32).T
    iaT = np.asarray(ia_interact_mat, dtype=np.float32).T
    uaT_dev = np.zeros((nc_, CHANNEL, up), dtype=np.float32)
    iaT_dev = np.zeros((nc_, CHANNEL, ip), dtype=np.float32)
    for c in range(nc_):
        uaT_dev[c, :, :upc] = uaT[:, c * upc:(c + 1) * upc]
        iaT_dev[c, :, :ipc] = iaT[:, c * ipc:(c + 1) * ipc]

    iota = np.broadcast_to(np.arange(128, dtype=np.float32), (128, 128)).copy()
    # extended weight table: w2[r, 0:64] = weight[r] (tail even),
    #                        w2[16+r, 64:128] = weight[r] (tail odd)
    # hi+lo fp16 split of the fp32 weights: w ~= w_hi + w_lo exactly enough
    wf = np.asarray(weight, dtype=np.float32)
    w_hi = wf.astype(np.float16)
    w_lo = (wf - w_hi.astype(np.float32)).astype(np.float16)
    w2 = np.zeros((2, 2 * N_REL, 2 * CHANNEL), dtype=np.float16)
    for i, wc in enumerate((w_hi, w_lo)):
        w2[i, :N_REL, :CHANNEL] = wc
        w2[i, N_REL:, CHANNEL:] = wc
    aspect2 = (2.0 * np.asarray(aspect_emb, dtype=np.float32))
    # entity table as pair rows [npair(+pad), 128]
    ent_pairs = np.zeros((nseg * seg_sz, 2 * CHANNEL), dtype=np.float32)
    ent_pairs[:npair] = np.asarray(entity_emb, dtype=np.float32).reshape(npair, 2 * CHANNEL)

    in_maps = []
    for c in range(nc_):
        in_maps.append({
            "entp": ent_pairs,
            "idx16": np.ascontiguousarray(idx_dev[c]),
            "wvals": wv_dev[c],
            "oneT": one_dev[c],
            "recip": recip_dev[c],
            "iota": iota,
            "w2": w2,
            "aspect2": aspect2,
            "uaT": uaT_dev[c],
            "iaT": iaT_dev[c],
        })
    meta = dict(L=L, ent_of_slot=ent_of_slot, cfg=cfg)
    return in_maps, meta


# ---------------------------------------------------------------------------
# device program
# ---------------------------------------------------------------------------

def build_program(cfg, L, reps=1, parts=("gather", "compute", "aux")):
    import concourse.bacc as bacc
    import concourse.mybir as mybir
    from concourse.tile import TileContext

    f32 = mybir.dt.float32
    f16 = mybir.dt.float16
    i16 = mybir.dt.int16

    seg_sz = cfg["seg"]
    nseg = cfg["nseg"]
    tpc, bt, nb = cfg["tpc"], cfg["bt"], cfg["nb"]
    tu, ti, ua_grp = cfg["tu"], cfg["ti"], cfg["ua_grp"]
    sub = cfg["sub"]
    rnt = L * 128
    rn = bt * rnt
    epad = nb * nseg * bt * rnt
    nch = epad // 128
    up, ip = tu * 128, ti * 128
    C2 = 2 * CHANNEL

    nc = bacc.Bacc(None, target_bir_lowering=False, debug=False,
                   dynamic_dma_scratch_size=cfg.get("dma_scratch", 16384))
    entp = nc.declare_dram_parameter("entp", [nseg * seg_sz, C2], f32, isOutput=False)
    idx16 = nc.declare_dram_parameter("idx16", [nb, nseg, 128, rn // 16], i16, isOutput=False)
    wvals = nc.declare_dram_parameter("wvals", [128, nch], f32, isOutput=False)
    oneT = nc.declare_dram_parameter("oneT", [2 * N_REL, epad], f16, isOutput=False)
    recip = nc.declare_dram_parameter("recip", [128, tpc], f32, isOutput=False)
    iota = nc.declare_dram_parameter("iota", [128, 128], f32, isOutput=False)
    w2 = nc.declare_dram_parameter("w2", [2, 2 * N_REL, C2], f16, isOutput=False)
    aspect2 = nc.declare_dram_parameter("aspect2", [CHANNEL, CHANNEL], f32, isOutput=False)
    uaT = nc.declare_dram_parameter("uaT", [CHANNEL, up], f32, isOutput=False)
    iaT = nc.declare_dram_parameter("iaT", [CHANNEL, ip], f32, isOutput=False)
    ent_out = nc.declare_dram_parameter("ent_out", [tpc * 128, CHANNEL], f32, isOutput=True)
    ua_out = nc.declare_dram_parameter("ua_out", [up, CHANNEL], f32, isOutput=True)
    ia_out = nc.declare_dram_parameter("ia_out", [ip, CHANNEL], f32, isOutput=True)

    is_eq = mybir.AluOpType.is_equal
    mult = mybir.AluOpType.mult
    add = mybir.AluOpType.add

    ua_blocks = [("ua", g) for g in range(tu // ua_grp)]
    ia_blocks = [("ia", g) for g in range(ti // ua_grp)]
    aux = ua_blocks + ia_blocks

    with TileContext(nc) as tc:
        with (
            tc.tile_pool(name="const", bufs=1) as constp,
            tc.tile_pool(name="gbuf", bufs=3) as gbufp,
            tc.tile_pool(name="idxp", bufs=3) as idxp,
            tc.tile_pool(name="onep", bufs=3) as onep,
            tc.tile_pool(name="wvp", bufs=3) as wvp,
            tc.tile_pool(name="mbuf", bufs=4) as mbufp,
            tc.tile_pool(name="msgb", bufs=4) as msgp,
            tc.tile_pool(name="foldb", bufs=3) as foldp,
            tc.tile_pool(name="outp", bufs=2) as outp,
            tc.tile_pool(name="uain", bufs=2) as uainp,
            tc.tile_pool(name="uast", bufs=2) as uastp,
            tc.tile_pool(name="psrel", bufs=4, space="PSUM") as psrelp,
            tc.tile_pool(name="pssum", bufs=2, space="PSUM") as pssump,
            tc.tile_pool(name="psua", bufs=2, space="PSUM") as psuap,
        ):
            iota_sb = constp.tile([128, 128], f32)
            nc.sync.dma_start(out=iota_sb[:], in_=iota[:])
            w2_sb = constp.tile([2 * N_REL, 2, C2], f16)
            for i in range(2):
                nc.sync.dma_start(out=w2_sb[:, i, :], in_=w2[i])
            asp_sb = constp.tile([CHANNEL, CHANNEL], f32)
            nc.sync.dma_start(out=asp_sb[:], in_=aspect2[:])
            recip_sb = constp.tile([128, tpc], f32)
            nc.sync.dma_start(out=recip_sb[:], in_=recip[:])

            def emit_aux(kind, g):
                src = uaT if kind == "ua" else iaT
                dst = ua_out if kind == "ua" else ia_out
                cols = ua_grp * 128
                in_t = uainp.tile([CHANNEL, cols], f32, tag="uain")
                nc.sync.dma_start(out=in_t[:], in_=src[:, g * cols:(g + 1) * cols])
                stage = uastp.tile([128, ua_grp, CHANNEL], f32, tag="uast")
                for t in range(ua_grp):
                    ps = psuap.tile([128, CHANNEL], f32)
                    nc.tensor.matmul(
                        out=ps[:],
                        lhsT=in_t[:, t * 128:(t + 1) * 128],
                        rhs=asp_sb[:],
                        start=True, stop=True,
                    )
                    nc.vector.tensor_copy(out=stage[:, t, :], in_=ps[:])
                dview = dst[g * cols:(g + 1) * cols, :].rearrange(
                    "(a p) c -> p a c", p=128)
                nc.sync.dma_start(out=dview, in_=stage[:])

            n_aux_total = len(aux) * reps
            n_blk_total = nb * reps
            for rep_b in range(n_blk_total):
                b = rep_b % nb
                g_tile = gbufp.tile([128, nseg * bt * L, C2], f32, tag="gt")
                idx_t = idxp.tile([128, nseg, rn // 16], i16, tag="ix")
                one_t = onep.tile([2 * N_REL, nseg * bt * rnt], f16, tag="oh")
                wv_t = wvp.tile([128, nseg * bt * L], f32, tag="wv")
                for s in range(nseg):
                    nc.sync.dma_start(out=idx_t[:, s, :], in_=idx16[b, s])
                nc.sync.dma_start(out=one_t[:], in_=oneT[:, b * rn * nseg:(b + 1) * rn * nseg])
                nc.sync.dma_start(out=wv_t[:], in_=wvals[:, b * nseg * bt * L:(b + 1) * nseg * bt * L])
                for s in range(nseg if "gather" in parts else 0):
                    lo = s * seg_sz
                    for h in range(bt):
                        # split each (seg, tile) gather into <=512-descriptor
                        # calls so two calls fit in the 1024-slot SWDGE ring
                        # (desc-gen of call N+1 overlaps the drain of call N)
                        c0 = 0
                        while c0 < L:
                            cn = min(4, L - c0)
                            base = (s * bt + h) * L + c0
                            nc.gpsimd.dma_gather(
                                out_ap=g_tile[:, base:base + cn, :],
                                in_ap=entp[lo:lo + seg_sz, :],
                                idxs_ap=idx_t[:, s, (h * rnt + c0 * 128) // 16:(h * rnt + (c0 + cn) * 128) // 16],
                                num_idxs=cn * 128,
                                num_idxs_reg=cn * 128,
                                elem_size=C2,
                                single_packet=False,
                            )
                            c0 += cn
                for t in range(bt if "compute" in parts else 0):
                    ps_sum = pssump.tile([128, C2], f32)
                    for s in range(nseg):
                        g0 = s * bt * L + t * L
                        for c0 in range(0, L, sub):
                            nchk = min(sub, L - c0)
                            ps_rel = psrelp.tile([128, sub * C2], f32, tag="psrel")
                            m_t = mbufp.tile([128, sub, 128], f32, tag="m")
                            msg_t = msgp.tile([128, sub, C2], f32, tag="msg")
                            nc.any.tensor_tensor(
                                out=m_t[:, :nchk, :],
                                in0=wv_t[:, g0 + c0:g0 + c0 + nchk].unsqueeze(2).broadcast_to([128, nchk, 128]),
                                in1=iota_sb[:].unsqueeze(1).broadcast_to([128, nchk, 128]),
                                op=is_eq,
                            )
                            for c in range(nchk):
                                for i in range(2):
                                    nc.tensor.matmul(
                                        out=ps_rel[:, c * C2:(c + 1) * C2],
                                        lhsT=one_t[:, (g0 + c0 + c) * 128:(g0 + c0 + c + 1) * 128],
                                        rhs=w2_sb[:, i, :],
                                        start=(i == 0), stop=(i == 1),
                                        skip_group_check=True,
                                    )
                            nc.vector.tensor_tensor(
                                out=msg_t[:, :nchk, :],
                                in0=g_tile[:, g0 + c0:g0 + c0 + nchk, :],
                                in1=ps_rel[:, :nchk * C2].rearrange("p (l c) -> p l c", c=C2),
                                op=mult,
                            )
                            for c in range(nchk):
                                nc.tensor.matmul(
                                    out=ps_sum[:],
                                    lhsT=m_t[:, c, :],
                                    rhs=msg_t[:, c, :],
                                    start=(s == 0 and c0 + c == 0),
                                    stop=(s == nseg - 1 and c0 + c == L - 1),
                                    skip_group_check=True,
                                )
                    tg = b * bt + t
                    o_col = tg % ua_grp
                    if o_col == 0:
                        o_stage = outp.tile([128, ua_grp, CHANNEL], f32, tag="eo")
                    fold_t = foldp.tile([128, CHANNEL], f32, tag="fold")
                    nc.vector.tensor_scalar(
                        out=fold_t[:],
                        in0=ps_sum[:, :CHANNEL],
                        scalar1=recip_sb[:, tg:tg + 1],
                        scalar2=None,
                        op0=mult,
                    )
                    nc.vector.scalar_tensor_tensor(
                        out=o_stage[:, o_col, :],
                        in0=ps_sum[:, CHANNEL:],
                        scalar=recip_sb[:, tg:tg + 1],
                        in1=fold_t[:],
                        op0=mult,
                        op1=add,
                    )
                    if o_col == ua_grp - 1:
                        blk0 = tg - (ua_grp - 1)
                        oview = ent_out[blk0 * 128:(tg + 1) * 128, :].rearrange(
                            "(a p) c -> p a c", p=128)
                        nc.sync.dma_start(out=oview, in_=o_stage[:])
                for j in range(rep_b * n_aux_total // n_blk_total,
                               (rep_b + 1) * n_aux_total // n_blk_total):
                    if "aux" in parts:
                        emit_aux(*aux[j % len(aux)])

    nc.compile()
    return nc


# ---------------------------------------------------------------------------
# entry point
# ---------------------------------------------------------------------------

def kernel(entity_emb, item_emb, user_emb, aspect_emb, edge_index, edge_type,
           ua_interact_mat, ia_interact_mat, weight, _cfg=None, _run_opts=None):
    from concourse.bass_utils import run_bass_kernel_spmd

    cfg = _cfg or full_cfg()
    in_maps, meta = host_prep(
        entity_emb, item_emb, user_emb, aspect_emb, edge_index, edge_type,
        ua_interact_mat, ia_interact_mat, weight, cfg)
    nc = build_program(cfg, meta["L"])

    res = run_bass_kernel_spmd(nc, in_maps, list(range(cfg["n_cores"])),
                               **(_run_opts or {}))
    outs = res.results

    nc_ = cfg["n_cores"]
    ne, nu, ni = cfg["n_entities"], cfg["n_users"], cfg["n_items"]
    upc, ipc = nu // nc_, ni // nc_
    ent_of_slot = meta["ent_of_slot"]

    ent_stack = np.concatenate([outs[c]["ent_out"] for c in range(nc_)], axis=0)
    entity_agg = np.zeros((ne, CHANNEL), dtype=np.float32)
    valid = ent_of_slot >= 0
    entity_agg[ent_of_slot[valid]] = ent_stack[valid]

    user_agg = np.concatenate([outs[c]["ua_out"][:upc] for c in range(nc_)], axis=0)
    item_agg = np.concatenate([outs[c]["ia_out"][:ipc] for c in range(nc_)], axis=0)

    kernel._last_results = res
    return (item_agg, entity_agg, user_agg)
